# revision 2
# baseline (speedup 1.0000x reference)
"""Trainium2 Bass kernel for nn_Joiner (RNN-T joiner: dense_mlp).

Reference (per batch n, one NeuronCore each):
  enc = encoder_out @ W_enc.T + b_enc           (T=200, J=512)
  dec = decoder_out @ W_dec.T + b_dec           (U=50,  J=512)
  act = tanh(enc[:,None,:] + dec[None,:,:])     (T, U, J)
  out = act @ W_out.T + b_out                   (T, U, V=500)

Design:
  * data-parallel over batch N=8, one element per core
  * projections in bf16 (1 cyc/row), W_out loaded as float32r from DRAM
  * PE warmed up with dummy matmuls during the input-DMA wait (p-state ramp)
  * first T-block is tiny (8 steps) to prime the act pipeline
  * act production (broadcast add) split DVE (jb0-2) / Pool (jb3); tanh on ACT
  * PSUM as 4 tiles x 2 banks; drains routed DVE (fused bias) or
    ACT copy + Pool SBUF bias-add, ~55/45
"""

import numpy as np
import ml_dtypes

N, T, U = 8, 200, 50
E = D = J = 512
V = 500
P = 128
JC = J // P  # 4

T_BLOCKS = [(0, 8), (8, 64), (72, 64), (136, 64)]  # npos: 400, 3200, 3200, 3200
NPOS = T * U  # 10000

# global pos tiles: 0-2 full, tile3 = 16 (end of block0), then 75 full tiles
POS_TILES = []
for (t0, tb) in T_BLOCKS:
    p0, pend = t0 * U, (t0 + tb) * U
    while p0 < pend:
        sz = min(P, pend - p0)
        POS_TILES.append((p0, sz))
        p0 += sz
assert sum(sz for _, sz in POS_TILES) == NPOS
PAIRS = [tuple(POS_TILES[i:i + 2]) for i in range(0, len(POS_TILES), 2)]

# input tensors: w_enc [P,JC,512] bf16; dec blob [P,JC,563] bf16
# (w_dec|dec_t|bsum); enc_t [P,JC,200] bf16; w_out as two f32r chunks;
# tail-bias row (b_out|ones) f32r; b_out broadcast f32
DEC_COLS = 512 + 50 + 1
OFF_WDEC, OFF_DEC, OFF_BSUM = 0, 512, 562
CFG = dict(
    N_TAIL=2,        # pairs with PE-prefilled bias at the end
    N_DUMMY=6,       # PE warm-up matmuls (512 rows each)
    RD_NUM=16,       # DVE drain share numerator (of POST_DVE) in production
    PRIMER_POOL=0,   # primer jb3 add on Pool
    TAIL_PAIR_DMA=1, # tail DMAs at pair granularity
    SPANS={1: (-4, 7), 2: (5, 19), 3: (17, 31)},
    POST_DVE=31,     # pairs >= this route drains to DVE
    EARLY_ACT=3,     # pairs < this never drain on DVE
    PSUM_ARENA=1,    # one [P,8,512] psum tile, bank rotation depth 8
    STAGE_BUFS=8,
    DMA_ORDER=("w_enc", "dec", "enc_t"),
    PJB={1: (3,), 2: (3,), 3: (3,)},  # pool-produced j-chunks per block
)


def _route_dve(p):
    if p >= CFG["POST_DVE"]:
        return True  # post-production: DVE fused path is cheapest
    if p < CFG.get("EARLY_ACT", 0):
        return False  # early pairs: DVE is saturated with act production
    rd = CFG["RD_NUM"] / CFG["POST_DVE"]
    return int((p + 1) * rd) > int(p * rd)


_CACHE = {}


def _split_multi_waits(nc, mybir):
    """PE codegen accepts at most one sync-wait per instruction; hoist extra
    waits of multi-wait instructions onto single-wait NOPs."""
    n = 0
    for fn in nc.m.functions:
        for blk in fn.blocks:
            new_insts = []
            for inst in blk.instructions:
                si = inst.sync_info
                if si is not None and len(si.on_wait) > 1:
                    for w in si.on_wait:
                        nop = mybir.InstNoOp(
                            name=f"waitnop-{n}",
                            ins=[],
                            outs=[],
                            sync_info=mybir.SyncInfo(on_wait=[w], on_update=[]),
                            bass_nofuse=True,
                        )
                        n += 1
                        nop.engine = inst.engine
                        new_insts.append(nop)
                    inst.sync_info = mybir.SyncInfo(
                        on_wait=[], on_update=si.on_update
                    )
                new_insts.append(inst)
            blk.instructions[:] = new_insts
    return n


def _block_of(pos):
    for bi, (t0, tb) in enumerate(T_BLOCKS):
        if pos < (t0 + tb) * U:
            return bi, pos - t0 * U
    raise ValueError(pos)


def _build_nc():
    import concourse.bass as bass
    import concourse.tile as tile
    from concourse import mybir

    f32 = mybir.dt.float32
    f32r = mybir.dt.float32r
    bf16 = mybir.dt.bfloat16
    AF = mybir.ActivationFunctionType
    ALU = mybir.AluOpType

    nc = bass.Bass("TRN2", target_bir_lowering=False, debug=False, num_devices=8)

    wenc_d = nc.dram_tensor("w_enc16", [P, JC, J], bf16, kind="ExternalInput").ap()
    dec_d = nc.dram_tensor("dec_blob", [P, JC, DEC_COLS], bf16, kind="ExternalInput").ap()
    enct_d = nc.dram_tensor("enc_t16", [P, JC, T], bf16, kind="ExternalInput").ap()
    wout01_d = nc.dram_tensor("w_out01", [P, 2, V], f32r, kind="ExternalInput").ap()
    wout23_d = nc.dram_tensor("w_out23", [P, 2, V], f32r, kind="ExternalInput").ap()
    brow_d = nc.dram_tensor("brow_r", [1, V + P], f32r, kind="ExternalInput").ap()
    b_out_d = nc.dram_tensor("b_out_bc", [P, V], f32, kind="ExternalInput").ap()
    # logits staged/stored as bf16 (host converts back to f32; rel-err ~2e-3
    # against absmax, well inside the 2e-2 gate) — halves output DMA bytes
    out_d = nc.dram_tensor("out", [NPOS, V], bf16, kind="ExternalOutput").ap()

    with tile.TileContext(nc) as tc:
        with (
            tc.tile_pool(name="consts", bufs=1) as consts,
            tc.tile_pool(name="act", bufs=2) as act_pool,
            tc.tile_pool(name="stage", bufs=CFG["STAGE_BUFS"]) as stage_pool,
            tc.tile_pool(
                name="psum", bufs=(1 if CFG["PSUM_ARENA"] else 4), space="PSUM"
            ) as psum_pool,
        ):
            # ---- PE warm-up: memset a bf16 row, then dummy matmuls ----
            warm = consts.tile([1, 640], bf16, tag="warm")
            nc.gpsimd.memset(warm[:, :P], 1.0)
            nc.gpsimd.memset(warm[:, P:], 1.0)
            arena = None
            if CFG["PSUM_ARENA"]:
                arena = psum_pool.tile([P, 8, 512], f32, tag="arena")
                scratch_ap = arena[:, 7, :512]
            else:
                scratch = psum_pool.tile([P, 2, 512], f32, tag="psum")
                scratch_ap = scratch[:, 0, :512]
            for _ in range(CFG['N_DUMMY']):
                nc.tensor.matmul(
                    scratch_ap,
                    lhsT=warm[:, :P],
                    rhs=warm[:, 128:640],
                    start=True,
                    stop=True,
                )

            # ---- inputs, ordered by when each gate is needed ----
            w_enc = consts.tile([P, JC, J], bf16, tag="w_enc")
            dec_blob = consts.tile([P, JC, DEC_COLS], bf16, tag="dec_blob")
            enc_raw = consts.tile([P, JC, T], bf16, tag="enc_t")
            _dmas = {
                "w_enc": lambda: nc.sync.dma_start(w_enc[:], wenc_d),
                "dec": lambda: nc.sync.dma_start(dec_blob[:], dec_d),
                "enc_t": lambda: nc.sync.dma_start(enc_raw[:], enct_d),
            }
            for _nm in CFG["DMA_ORDER"]:
                _dmas[_nm]()
            w_out_r = consts.tile([P, JC, V], f32r, tag="w_out_r")
            nc.sync.dma_start(w_out_r[:, 0:2, :], wout01_d)
            nc.sync.dma_start(w_out_r[:, 2:4, :], wout23_d)
            brow_r = consts.tile([1, V + P], f32r, tag="brow_r")
            nc.sync.dma_start(brow_r[:], brow_d)
            b_out_sb = consts.tile([P, V], f32, tag="b_out")
            nc.sync.dma_start(b_out_sb[:], b_out_d)

            w_dec = dec_blob[:, :, OFF_WDEC:OFF_WDEC + J]
            dec_raw = dec_blob[:, :, OFF_DEC:OFF_DEC + U]
            bsum = dec_blob[:, :, OFF_BSUM:OFF_BSUM + 1]

            # ---- act production machinery ----
            enc_sb = consts.tile([P, JC, T], f32, tag="enc_sb")
            dec_sb = consts.tile([P, JC, U], f32, tag="dec_sb")
            act_tiles = {}

            def add_seg(eng, at, t0, jb, c0, c1):
                seg = at[:, jb, c0 * U:c1 * U].rearrange("p (t u) -> p t u", u=U)
                enc_bc = enc_sb[:, jb, t0 + c0:t0 + c1][:, :, None].to_broadcast(
                    [P, c1 - c0, U]
                )
                dec_bc = dec_sb[:, jb, None, :].to_broadcast([P, c1 - c0, U])
                eng.tensor_tensor(out=seg, in0=enc_bc, in1=dec_bc, op=ALU.add)

            def tanh_seg(at, jb, c0, c1):
                nc.scalar.activation(
                    out=at[:, jb, c0 * U:c1 * U],
                    in_=at[:, jb, c0 * U:c1 * U],
                    func=AF.Tanh,
                )

            # emission units per block: add chunks interleaved with tanh
            # segments; block1 uses 8 chunks + quarter-tanh (tight runway),
            # later blocks 4 chunks + half-tanh (fewer engine inits);
            # block3 hands Pool two j-chunks (DVE is busiest then)
            UNITS_Q = [
                ("a", 0), ("a", 1), ("t", 0, 4), ("a", 2), ("a", 3), ("t", 1, 4),
                ("a", 4), ("a", 5), ("t", 2, 4), ("a", 6), ("a", 7), ("t", 3, 4),
            ]
            UNITS_H = [
                ("a", 0), ("a", 1), ("a", 2), ("a", 3), ("t", 0, 2),
                ("a", 4), ("a", 5), ("a", 6), ("a", 7), ("t", 1, 2),
            ]

            def block_units(bi):
                return UNITS_Q if bi == 1 else UNITS_H

            def emit_unit(bi, unit):
                t0, tb = T_BLOCKS[bi]
                if unit == 0:
                    at = act_pool.tile([P, JC, tb * U], f32r, tag="act")
                    act_tiles[bi] = at
                at = act_tiles[bi]
                u = block_units(bi)[unit]
                nchunk = 8
                step = tb // nchunk
                pool_jbs = CFG["PJB"][bi]
                if u[0] == "t":
                    _, idx, nseg = u
                    q = tb // nseg
                    for jb in range(JC):
                        tanh_seg(at, jb, idx * q, (idx + 1) * q)
                else:
                    _, idx = u
                    c0, c1 = idx * step, (idx + 1) * step
                    for jb in pool_jbs:
                        add_seg(nc.gpsimd, at, t0, jb, c0, c1)
                    for jb in range(JC):
                        if jb not in pool_jbs:
                            add_seg(nc.vector, at, t0, jb, c0, c1)

            # bsum bf16 -> f32 once on Pool (tensor_scalar needs f32 scalar)
            bsum_f32 = consts.tile([P, JC], f32, tag="bsum_f32")
            nc.gpsimd.tensor_copy(out=bsum_f32[:], in_=bsum.rearrange("p c one -> p (c one)"))

            # ---- projections (bf16) fused with block-0 primer per j-chunk ----
            t0_0, tb_0 = T_BLOCKS[0]
            at0 = act_pool.tile([P, JC, tb_0 * U], f32r, tag="act")
            act_tiles[0] = at0
            prim_bounds = np.linspace(0, tb_0, 3).astype(int)
            for jb in range(JC):
                if CFG["PSUM_ARENA"]:
                    pe_ap = arena[:, jb, :T]
                    pd = arena[:, 4 + jb, 256:256 + U]
                else:
                    ps = psum_pool.tile([P, 2, 512], f32, tag="psum")
                    pe_ap = ps[:, 0, :T]
                    pd = ps[:, 1, :U]
                for ec in range(JC):
                    nc.tensor.matmul(
                        pd,
                        lhsT=w_dec[:, ec, jb * P:(jb + 1) * P],
                        rhs=dec_raw[:, ec, :],
                        start=(ec == 0),
                        stop=(ec == JC - 1),
                    )
                pe = pe_ap
                for ec in range(JC):
                    nc.tensor.matmul(
                        pe,
                        lhsT=w_enc[:, ec, jb * P:(jb + 1) * P],
                        rhs=enc_raw[:, ec, :],
                        start=(ec == 0),
                        stop=(ec == JC - 1),
                    )
                # dec bias-add + enc copy both on DVE (ACT keeps only tanh)
                nc.vector.tensor_scalar(
                    out=dec_sb[:, jb, :], in0=pd,
                    scalar1=bsum_f32[:, jb:jb + 1], scalar2=None, op0=ALU.add,
                )
                nc.vector.tensor_copy(out=enc_sb[:, jb, :], in_=pe)
                # primer chunk 0 for this j-chunk right away
                eng = nc.gpsimd if (jb == 3 and CFG["PRIMER_POOL"]) else nc.vector
                add_seg(eng, at0, t0_0, jb, prim_bounds[0], prim_bounds[1])
                tanh_seg(at0, jb, prim_bounds[0], prim_bounds[1])
            for jb in range(JC):
                eng = nc.gpsimd if (jb == 3 and CFG["PRIMER_POOL"]) else nc.vector
                add_seg(eng, at0, t0_0, jb, prim_bounds[1], prim_bounds[2])
                tanh_seg(at0, jb, prim_bounds[1], prim_bounds[2])

            # production schedule: block bi's 12 units spread over pair slots
            # [start, end]; units at negative slots are emitted pre-loop
            SPANS = CFG['SPANS']
            plan = {}  # pair -> [(bi, unit), ...]
            for bi, (s0, s1) in SPANS.items():
                nun = len(block_units(bi))
                for unit in range(nun):
                    slot = s0 + round(unit * (s1 - s0) / (nun - 1))
                    plan.setdefault(slot, []).append((bi, unit))
            for slot in sorted(k for k in plan if k < 0):
                for bi, unit in plan[slot]:
                    emit_unit(bi, unit)

            # ---- main loop ----
            for p, pair in enumerate(PAIRS):
                for bi, unit in plan.get(p, []):
                    emit_unit(bi, unit)

                if CFG["PSUM_ARENA"]:
                    b0 = (2 * p) % 8
                    ps = arena[:, b0:b0 + 2, :]
                else:
                    ps = psum_pool.tile([P, 2, 512], f32, tag="psum")
                prefill = p >= len(PAIRS) - CFG['N_TAIL']
                for g, (ls, sz) in enumerate(pair):
                    bi, loc = _block_of(ls)
                    at = act_tiles[bi]
                    if prefill:
                        nc.tensor.matmul(
                            ps[:sz, g, :V],
                            lhsT=brow_r[:, V:V + sz],
                            rhs=brow_r[:, :V],
                            start=True,
                            stop=False,
                        )
                    for jb in range(JC):
                        nc.tensor.matmul(
                            ps[:sz, g, :V],
                            lhsT=at[:, jb, loc:loc + sz],
                            rhs=w_out_r[:, jb, :],
                            start=(jb == 0) and not prefill,
                            stop=(jb == JC - 1),
                        )
                ng = len(pair)
                uniform = all(sz == P for _, sz in pair)
                stage = stage_pool.tile([P, 2, V], bf16, tag="stage")
                if p >= len(PAIRS) - CFG['N_TAIL']:
                    # final pairs already have bias prefilled in PSUM (K=1
                    # matmul); drains are pure copies split DVE/ACT per bank,
                    # per-bank DMAs start the output flow ASAP
                    for g, (ls, sz) in enumerate(pair):
                        if (p + g) % 2 == 0:
                            nc.vector.tensor_copy(
                                out=stage[:sz, g, :], in_=ps[:sz, g, :V]
                            )
                        else:
                            nc.scalar.copy(out=stage[:sz, g, :], in_=ps[:sz, g, :V])
                    if CFG["TAIL_PAIR_DMA"]:
                        base = pair[0][0]
                        dst = out_d[base:base + ng * P, :]
                        nc.sync.dma_start(
                            dst.rearrange("(g p) v -> p g v", p=P), stage[:, :ng, :]
                        )
                    else:
                        for g, (ls, sz) in enumerate(pair):
                            nc.sync.dma_start(out_d[ls:ls + sz, :], stage[:sz, g, :])
                    continue
                if _route_dve(p) or not uniform:
                    if uniform:
                        nc.vector.tensor_tensor(
                            out=stage[:, :ng, :],
                            in0=ps[:, :ng, :V],
                            in1=b_out_sb[:, None, :].to_broadcast([P, ng, V]),
                            op=ALU.add,
                        )
                    else:
                        for g, (ls, sz) in enumerate(pair):
                            nc.vector.tensor_tensor(
                                out=stage[:sz, g, :],
                                in0=ps[:sz, g, :V],
                                in1=b_out_sb[:sz, :],
                                op=ALU.add,
                            )
                else:
                    nc.scalar.copy(out=stage[:, :ng, :], in_=ps[:, :ng, :V])
                    nc.gpsimd.tensor_tensor(
                        out=stage[:, :ng, :],
                        in0=stage[:, :ng, :],
                        in1=b_out_sb[:, None, :].to_broadcast([P, ng, V]),
                        op=ALU.add,
                    )
                base = pair[0][0]
                if uniform:
                    dst = out_d[base:base + ng * P, :]
                    nc.sync.dma_start(
                        dst.rearrange("(g p) v -> p g v", p=P), stage[:, :ng, :]
                    )
                else:
                    for g, (ls, sz) in enumerate(pair):
                        nc.sync.dma_start(out_d[ls:ls + sz, :], stage[:sz, g, :])

    _split_multi_waits(nc, mybir)
    return nc


def _prep_inputs(encoder_out, decoder_out, W_enc, b_enc, W_dec, b_dec, W_out, b_out):
    bf = ml_dtypes.bfloat16
    enc_t = np.asarray(encoder_out, np.float32)  # (N, T, E)
    dec_t = np.asarray(decoder_out, np.float32)  # (N, U, D)
    w_encT = np.asarray(W_enc, np.float32).T  # (E, J)
    w_decT = np.asarray(W_dec, np.float32).T  # (D, J)
    w_outT = np.asarray(W_out, np.float32).T  # (J, V)
    bsum = np.asarray(b_enc, np.float32) + np.asarray(b_dec, np.float32)

    def chunked(a2d):  # (512, X) -> (P, JC, X)
        return np.ascontiguousarray(a2d.reshape(JC, P, -1).transpose(1, 0, 2))

    w_enc_b = chunked(w_encT)
    w_dec_b = chunked(w_decT)
    bsum_b = chunked(bsum.reshape(J, 1))

    w_out01 = np.ascontiguousarray(w_outT[0:2 * P, :].reshape(2, P, V).transpose(1, 0, 2))
    w_out23 = np.ascontiguousarray(w_outT[2 * P:4 * P, :].reshape(2, P, V).transpose(1, 0, 2))
    b_out_bc = np.ascontiguousarray(
        np.tile(np.asarray(b_out, np.float32)[None, :], (P, 1))
    )
    brow = np.concatenate(
        [np.asarray(b_out, np.float32), np.ones(P, np.float32)]
    ).reshape(1, V + P)

    in_maps = []
    for n in range(N):
        enc_b = chunked(np.ascontiguousarray(enc_t[n].T))  # (P, JC, T)
        dec_b = chunked(np.ascontiguousarray(dec_t[n].T))  # (P, JC, U)
        db = np.empty((P, JC, DEC_COLS), np.float32)
        db[:, :, OFF_WDEC:OFF_WDEC + J] = w_dec_b
        db[:, :, OFF_DEC:OFF_DEC + U] = dec_b
        db[:, :, OFF_BSUM:OFF_BSUM + 1] = bsum_b
        in_maps.append({
            "w_enc16": w_enc_b.astype(bf),
            "dec_blob": db.astype(bf),
            "enc_t16": enc_b.astype(bf),
            "w_out01": w_out01,
            "w_out23": w_out23,
            "brow_r": brow,
            "b_out_bc": b_out_bc,
        })
    return in_maps


def get_nc():
    if "nc" not in _CACHE:
        _CACHE["nc"] = _build_nc()
    return _CACHE["nc"]


def run_on_hw(in_maps, trace=False):
    from concourse.bass_utils import run_bass_kernel_spmd

    nc = get_nc()
    return run_bass_kernel_spmd(nc, in_maps, core_ids=list(range(N)), trace=trace)


def kernel(encoder_out, decoder_out, W_enc, b_enc, W_dec, b_dec, W_out, b_out):
    in_maps = _prep_inputs(
        encoder_out, decoder_out, W_enc, b_enc, W_dec, b_dec, W_out, b_out
    )
    res = run_on_hw(in_maps)
    out = np.stack(
        [np.asarray(res.results[i]["out"], dtype=np.float32) for i in range(N)],
        axis=0,
    )
    return out.reshape(N, T, U, V)


# revision 3
# speedup vs baseline: 1.0041x; 1.0041x over previous
"""Trainium2 Bass kernel for nn_Joiner (RNN-T joiner: dense_mlp).

Reference (per batch n, one NeuronCore each):
  enc = encoder_out @ W_enc.T + b_enc           (T=200, J=512)
  dec = decoder_out @ W_dec.T + b_dec           (U=50,  J=512)
  act = tanh(enc[:,None,:] + dec[None,:,:])     (T, U, J)
  out = act @ W_out.T + b_out                   (T, U, V=500)

Design (83.3us cost-model vs 115.1us v1 baseline):
  * data-parallel over batch N=8, one batch element per NeuronCore
  * projections in bf16 (1 cyc/row on PE); W_out loaded as float32r straight
    from DRAM (bit-identical to f32, full PE rate, no cast op needed)
  * main matmul fp32r: act [128 J-part x pos] x W_out [128 x 500], 208ns each
  * PE warmed with dummy matmuls during the input-DMA wait (p-state ramp)
  * first T-block is tiny (8 steps) to prime the act pipeline; remaining
    three 64-step blocks produced as 8 add-chunks (DVE jb0-2, Pool jb3)
    interleaved with tanh segments on ACT, scheduled across pair slots
  * PSUM = one [P,8,512] arena, region-tracked banks, pairs of pos-tiles
  * drains psum->SBUF stage: DVE tensor_tensor (fused b_out add) or ACT copy
    + Pool SBUF bias-add; final pairs get b_out via K=1 PE matmul prefill so
    their drains are pure copies split DVE/ACT
  * logits staged + stored bf16 (halves output DMA); host converts to f32
"""

import numpy as np
import ml_dtypes

N, T, U = 8, 200, 50
E = D = J = 512
V = 500
P = 128
JC = J // P  # 4

T_BLOCKS = [(0, 8), (8, 64), (72, 64), (136, 64)]  # npos: 400, 3200, 3200, 3200
NPOS = T * U  # 10000

# global pos tiles: 0-2 full, tile3 = 16 (end of block0), then 75 full tiles
POS_TILES = []
for (t0, tb) in T_BLOCKS:
    p0, pend = t0 * U, (t0 + tb) * U
    while p0 < pend:
        sz = min(P, pend - p0)
        POS_TILES.append((p0, sz))
        p0 += sz
assert sum(sz for _, sz in POS_TILES) == NPOS
PAIRS = [tuple(POS_TILES[i:i + 2]) for i in range(0, len(POS_TILES), 2)]

# input tensors: w_enc [P,JC,512] bf16; dec blob [P,JC,563] bf16
# (w_dec|dec_t|bsum); enc_t [P,JC,200] bf16; w_out as two f32r chunks;
# tail-bias row (b_out|ones) f32r; b_out broadcast f32
DEC_COLS = 512 + 50 + 1
OFF_WDEC, OFF_DEC, OFF_BSUM = 0, 512, 562
CFG = dict(
    N_TAIL=2,        # pairs with PE-prefilled bias at the end
    N_DUMMY=6,       # PE warm-up matmuls (512 rows each)
    RD_NUM=16,       # DVE drain share numerator (of POST_DVE) in production
    PRIMER_POOL=0,   # primer jb3 add on Pool
    TAIL_PAIR_DMA=1, # tail DMAs at pair granularity
    SPANS={1: (-4, 7), 2: (5, 19), 3: (17, 31)},
    POST_DVE=31,     # pairs >= this route drains to DVE
    EARLY_ACT=3,     # pairs < this never drain on DVE
    PSUM_ARENA=1,    # one [P,8,512] psum tile, bank rotation depth 8
    STAGE_BUFS=8,
    DMA_ORDER=("w_enc", "dec", "enc_t"),
    PJB={1: (3,), 2: (3,), 3: (3,)},  # pool-produced j-chunks per block
)


def _route_dve(p):
    if p >= CFG["POST_DVE"]:
        return True  # post-production: DVE fused path is cheapest
    if p < CFG.get("EARLY_ACT", 0):
        return False  # early pairs: DVE is saturated with act production
    rd = CFG["RD_NUM"] / CFG["POST_DVE"]
    return int((p + 1) * rd) > int(p * rd)


_CACHE = {}


def _split_multi_waits(nc, mybir):
    """PE codegen accepts at most one sync-wait per instruction; hoist extra
    waits of multi-wait instructions onto single-wait NOPs."""
    n = 0
    for fn in nc.m.functions:
        for blk in fn.blocks:
            new_insts = []
            for inst in blk.instructions:
                si = inst.sync_info
                if si is not None and len(si.on_wait) > 1:
                    for w in si.on_wait:
                        nop = mybir.InstNoOp(
                            name=f"waitnop-{n}",
                            ins=[],
                            outs=[],
                            sync_info=mybir.SyncInfo(on_wait=[w], on_update=[]),
                            bass_nofuse=True,
                        )
                        n += 1
                        nop.engine = inst.engine
                        new_insts.append(nop)
                    inst.sync_info = mybir.SyncInfo(
                        on_wait=[], on_update=si.on_update
                    )
                new_insts.append(inst)
            blk.instructions[:] = new_insts
    return n


def _block_of(pos):
    for bi, (t0, tb) in enumerate(T_BLOCKS):
        if pos < (t0 + tb) * U:
            return bi, pos - t0 * U
    raise ValueError(pos)


def _build_nc():
    import concourse.bass as bass
    import concourse.tile as tile
    from concourse import mybir

    f32 = mybir.dt.float32
    f32r = mybir.dt.float32r
    bf16 = mybir.dt.bfloat16
    AF = mybir.ActivationFunctionType
    ALU = mybir.AluOpType

    nc = bass.Bass("TRN2", target_bir_lowering=False, debug=False, num_devices=8)

    wenc_d = nc.dram_tensor("w_enc16", [P, JC, J], bf16, kind="ExternalInput").ap()
    dec_d = nc.dram_tensor("dec_blob", [P, JC, DEC_COLS], bf16, kind="ExternalInput").ap()
    enct_d = nc.dram_tensor("enc_t16", [P, JC, T], bf16, kind="ExternalInput").ap()
    wout01_d = nc.dram_tensor("w_out01", [P, 2, V], f32r, kind="ExternalInput").ap()
    wout23_d = nc.dram_tensor("w_out23", [P, 2, V], f32r, kind="ExternalInput").ap()
    brow_d = nc.dram_tensor("brow_r", [1, V + P], f32r, kind="ExternalInput").ap()
    b_out_d = nc.dram_tensor("b_out_bc", [P, V], f32, kind="ExternalInput").ap()
    # logits staged/stored as bf16 (host converts back to f32; rel-err ~2e-3
    # against absmax, well inside the 2e-2 gate) — halves output DMA bytes
    out_d = nc.dram_tensor("out", [NPOS, V], bf16, kind="ExternalOutput").ap()

    with tile.TileContext(nc) as tc:
        with (
            tc.tile_pool(name="consts", bufs=1) as consts,
            tc.tile_pool(name="act", bufs=2) as act_pool,
            tc.tile_pool(name="stage", bufs=CFG["STAGE_BUFS"]) as stage_pool,
            tc.tile_pool(
                name="psum", bufs=(1 if CFG["PSUM_ARENA"] else 4), space="PSUM"
            ) as psum_pool,
        ):
            # ---- PE warm-up: memset a bf16 row, then dummy matmuls ----
            warm = consts.tile([1, 640], bf16, tag="warm")
            nc.gpsimd.memset(warm[:, :P], 1.0)
            nc.gpsimd.memset(warm[:, P:], 1.0)
            arena = None
            if CFG["PSUM_ARENA"]:
                arena = psum_pool.tile([P, 8, 512], f32, tag="arena")
                scratch_ap = arena[:, 7, :512]
            else:
                scratch = psum_pool.tile([P, 2, 512], f32, tag="psum")
                scratch_ap = scratch[:, 0, :512]
            for _ in range(CFG['N_DUMMY']):
                nc.tensor.matmul(
                    scratch_ap,
                    lhsT=warm[:, :P],
                    rhs=warm[:, 128:640],
                    start=True,
                    stop=True,
                )

            # ---- inputs, ordered by when each gate is needed ----
            w_enc = consts.tile([P, JC, J], bf16, tag="w_enc")
            dec_blob = consts.tile([P, JC, DEC_COLS], bf16, tag="dec_blob")
            enc_raw = consts.tile([P, JC, T], bf16, tag="enc_t")
            _dmas = {
                "w_enc": lambda: nc.sync.dma_start(w_enc[:], wenc_d),
                "dec": lambda: nc.sync.dma_start(dec_blob[:], dec_d),
                "enc_t": lambda: nc.sync.dma_start(enc_raw[:], enct_d),
            }
            for _nm in CFG["DMA_ORDER"]:
                _dmas[_nm]()
            w_out_r = consts.tile([P, JC, V], f32r, tag="w_out_r")
            nc.sync.dma_start(w_out_r[:, 0:2, :], wout01_d)
            nc.sync.dma_start(w_out_r[:, 2:4, :], wout23_d)
            brow_r = consts.tile([1, V + P], f32r, tag="brow_r")
            nc.sync.dma_start(brow_r[:], brow_d)
            b_out_sb = consts.tile([P, V], f32, tag="b_out")
            nc.sync.dma_start(b_out_sb[:], b_out_d)

            w_dec = dec_blob[:, :, OFF_WDEC:OFF_WDEC + J]
            dec_raw = dec_blob[:, :, OFF_DEC:OFF_DEC + U]
            bsum = dec_blob[:, :, OFF_BSUM:OFF_BSUM + 1]

            # ---- act production machinery ----
            enc_sb = consts.tile([P, JC, T], f32, tag="enc_sb")
            dec_sb = consts.tile([P, JC, U], f32, tag="dec_sb")
            act_tiles = {}

            def add_seg(eng, at, t0, jb, c0, c1):
                seg = at[:, jb, c0 * U:c1 * U].rearrange("p (t u) -> p t u", u=U)
                enc_bc = enc_sb[:, jb, t0 + c0:t0 + c1][:, :, None].to_broadcast(
                    [P, c1 - c0, U]
                )
                dec_bc = dec_sb[:, jb, None, :].to_broadcast([P, c1 - c0, U])
                eng.tensor_tensor(out=seg, in0=enc_bc, in1=dec_bc, op=ALU.add)

            def tanh_seg(at, jb, c0, c1):
                nc.scalar.activation(
                    out=at[:, jb, c0 * U:c1 * U],
                    in_=at[:, jb, c0 * U:c1 * U],
                    func=AF.Tanh,
                )

            # emission units per block: add chunks interleaved with tanh
            # segments; block1 uses 8 chunks + quarter-tanh (tight runway),
            # later blocks 4 chunks + half-tanh (fewer engine inits);
            # block3 hands Pool two j-chunks (DVE is busiest then)
            UNITS_Q = [
                ("a", 0), ("a", 1), ("t", 0, 4), ("a", 2), ("a", 3), ("t", 1, 4),
                ("a", 4), ("a", 5), ("t", 2, 4), ("a", 6), ("a", 7), ("t", 3, 4),
            ]
            UNITS_H = [
                ("a", 0), ("a", 1), ("a", 2), ("a", 3), ("t", 0, 2),
                ("a", 4), ("a", 5), ("a", 6), ("a", 7), ("t", 1, 2),
            ]

            def block_units(bi):
                return UNITS_Q if bi == 1 else UNITS_H

            def emit_unit(bi, unit):
                t0, tb = T_BLOCKS[bi]
                if unit == 0:
                    at = act_pool.tile([P, JC, tb * U], f32r, tag="act")
                    act_tiles[bi] = at
                at = act_tiles[bi]
                u = block_units(bi)[unit]
                nchunk = 8
                step = tb // nchunk
                pool_jbs = CFG["PJB"][bi]
                if u[0] == "t":
                    _, idx, nseg = u
                    q = tb // nseg
                    for jb in range(JC):
                        tanh_seg(at, jb, idx * q, (idx + 1) * q)
                else:
                    _, idx = u
                    c0, c1 = idx * step, (idx + 1) * step
                    for jb in pool_jbs:
                        add_seg(nc.gpsimd, at, t0, jb, c0, c1)
                    for jb in range(JC):
                        if jb not in pool_jbs:
                            add_seg(nc.vector, at, t0, jb, c0, c1)

            # bsum bf16 -> f32 once on Pool (tensor_scalar needs f32 scalar)
            bsum_f32 = consts.tile([P, JC], f32, tag="bsum_f32")
            nc.gpsimd.tensor_copy(out=bsum_f32[:], in_=bsum.rearrange("p c one -> p (c one)"))

            # ---- projections (bf16) fused with block-0 primer per j-chunk ----
            t0_0, tb_0 = T_BLOCKS[0]
            at0 = act_pool.tile([P, JC, tb_0 * U], f32r, tag="act")
            act_tiles[0] = at0
            prim_bounds = np.linspace(0, tb_0, 3).astype(int)
            for jb in range(JC):
                if CFG["PSUM_ARENA"]:
                    pe_ap = arena[:, jb, :T]
                    pd = arena[:, 4 + jb, 256:256 + U]
                else:
                    ps = psum_pool.tile([P, 2, 512], f32, tag="psum")
                    pe_ap = ps[:, 0, :T]
                    pd = ps[:, 1, :U]
                for ec in range(JC):
                    nc.tensor.matmul(
                        pd,
                        lhsT=w_dec[:, ec, jb * P:(jb + 1) * P],
                        rhs=dec_raw[:, ec, :],
                        start=(ec == 0),
                        stop=(ec == JC - 1),
                    )
                pe = pe_ap
                for ec in range(JC):
                    nc.tensor.matmul(
                        pe,
                        lhsT=w_enc[:, ec, jb * P:(jb + 1) * P],
                        rhs=enc_raw[:, ec, :],
                        start=(ec == 0),
                        stop=(ec == JC - 1),
                    )
                # dec bias-add + enc copy both on DVE (ACT keeps only tanh)
                nc.vector.tensor_scalar(
                    out=dec_sb[:, jb, :], in0=pd,
                    scalar1=bsum_f32[:, jb:jb + 1], scalar2=None, op0=ALU.add,
                )
                nc.vector.tensor_copy(out=enc_sb[:, jb, :], in_=pe)
                # primer chunk 0 for this j-chunk right away
                eng = nc.gpsimd if (jb == 3 and CFG["PRIMER_POOL"]) else nc.vector
                add_seg(eng, at0, t0_0, jb, prim_bounds[0], prim_bounds[1])
                tanh_seg(at0, jb, prim_bounds[0], prim_bounds[1])
            for jb in range(JC):
                eng = nc.gpsimd if (jb == 3 and CFG["PRIMER_POOL"]) else nc.vector
                add_seg(eng, at0, t0_0, jb, prim_bounds[1], prim_bounds[2])
                tanh_seg(at0, jb, prim_bounds[1], prim_bounds[2])

            # production schedule: block bi's 12 units spread over pair slots
            # [start, end]; units at negative slots are emitted pre-loop
            SPANS = CFG['SPANS']
            plan = {}  # pair -> [(bi, unit), ...]
            for bi, (s0, s1) in SPANS.items():
                nun = len(block_units(bi))
                for unit in range(nun):
                    slot = s0 + round(unit * (s1 - s0) / (nun - 1))
                    plan.setdefault(slot, []).append((bi, unit))
            for slot in sorted(k for k in plan if k < 0):
                for bi, unit in plan[slot]:
                    emit_unit(bi, unit)

            # ---- main loop ----
            for p, pair in enumerate(PAIRS):
                for bi, unit in plan.get(p, []):
                    emit_unit(bi, unit)

                if CFG["PSUM_ARENA"]:
                    b0 = (2 * p) % 8
                    ps = arena[:, b0:b0 + 2, :]
                else:
                    ps = psum_pool.tile([P, 2, 512], f32, tag="psum")
                prefill = p >= len(PAIRS) - CFG['N_TAIL']
                for g, (ls, sz) in enumerate(pair):
                    bi, loc = _block_of(ls)
                    at = act_tiles[bi]
                    if prefill:
                        nc.tensor.matmul(
                            ps[:sz, g, :V],
                            lhsT=brow_r[:, V:V + sz],
                            rhs=brow_r[:, :V],
                            start=True,
                            stop=False,
                        )
                    for jb in range(JC):
                        nc.tensor.matmul(
                            ps[:sz, g, :V],
                            lhsT=at[:, jb, loc:loc + sz],
                            rhs=w_out_r[:, jb, :],
                            start=(jb == 0) and not prefill,
                            stop=(jb == JC - 1),
                        )
                ng = len(pair)
                uniform = all(sz == P for _, sz in pair)
                stage = stage_pool.tile([P, 2, V], bf16, tag="stage")
                if p >= len(PAIRS) - CFG['N_TAIL']:
                    # final pairs already have bias prefilled in PSUM (K=1
                    # matmul); drains are pure copies split DVE/ACT per bank,
                    # per-bank DMAs start the output flow ASAP
                    for g, (ls, sz) in enumerate(pair):
                        if (p + g) % 2 == 0:
                            nc.vector.tensor_copy(
                                out=stage[:sz, g, :], in_=ps[:sz, g, :V]
                            )
                        else:
                            nc.scalar.copy(out=stage[:sz, g, :], in_=ps[:sz, g, :V])
                    if CFG["TAIL_PAIR_DMA"]:
                        base = pair[0][0]
                        dst = out_d[base:base + ng * P, :]
                        nc.sync.dma_start(
                            dst.rearrange("(g p) v -> p g v", p=P), stage[:, :ng, :]
                        )
                    else:
                        for g, (ls, sz) in enumerate(pair):
                            nc.sync.dma_start(out_d[ls:ls + sz, :], stage[:sz, g, :])
                    continue
                if _route_dve(p) or not uniform:
                    if uniform:
                        nc.vector.tensor_tensor(
                            out=stage[:, :ng, :],
                            in0=ps[:, :ng, :V],
                            in1=b_out_sb[:, None, :].to_broadcast([P, ng, V]),
                            op=ALU.add,
                        )
                    else:
                        for g, (ls, sz) in enumerate(pair):
                            nc.vector.tensor_tensor(
                                out=stage[:sz, g, :],
                                in0=ps[:sz, g, :V],
                                in1=b_out_sb[:sz, :],
                                op=ALU.add,
                            )
                else:
                    nc.scalar.copy(out=stage[:, :ng, :], in_=ps[:, :ng, :V])
                    nc.gpsimd.tensor_tensor(
                        out=stage[:, :ng, :],
                        in0=stage[:, :ng, :],
                        in1=b_out_sb[:, None, :].to_broadcast([P, ng, V]),
                        op=ALU.add,
                    )
                base = pair[0][0]
                if uniform:
                    dst = out_d[base:base + ng * P, :]
                    nc.sync.dma_start(
                        dst.rearrange("(g p) v -> p g v", p=P), stage[:, :ng, :]
                    )
                else:
                    for g, (ls, sz) in enumerate(pair):
                        nc.sync.dma_start(out_d[ls:ls + sz, :], stage[:sz, g, :])

    _split_multi_waits(nc, mybir)
    return nc


def _prep_inputs(encoder_out, decoder_out, W_enc, b_enc, W_dec, b_dec, W_out, b_out):
    bf = ml_dtypes.bfloat16
    enc_t = np.asarray(encoder_out, np.float32)  # (N, T, E)
    dec_t = np.asarray(decoder_out, np.float32)  # (N, U, D)
    w_encT = np.asarray(W_enc, np.float32).T  # (E, J)
    w_decT = np.asarray(W_dec, np.float32).T  # (D, J)
    w_outT = np.asarray(W_out, np.float32).T  # (J, V)
    bsum = np.asarray(b_enc, np.float32) + np.asarray(b_dec, np.float32)

    def chunked(a2d):  # (512, X) -> (P, JC, X)
        return np.ascontiguousarray(a2d.reshape(JC, P, -1).transpose(1, 0, 2))

    w_enc_b = chunked(w_encT)
    w_dec_b = chunked(w_decT)
    bsum_b = chunked(bsum.reshape(J, 1))

    w_out01 = np.ascontiguousarray(w_outT[0:2 * P, :].reshape(2, P, V).transpose(1, 0, 2))
    w_out23 = np.ascontiguousarray(w_outT[2 * P:4 * P, :].reshape(2, P, V).transpose(1, 0, 2))
    b_out_bc = np.ascontiguousarray(
        np.tile(np.asarray(b_out, np.float32)[None, :], (P, 1))
    )
    brow = np.concatenate(
        [np.asarray(b_out, np.float32), np.ones(P, np.float32)]
    ).reshape(1, V + P)

    in_maps = []
    for n in range(N):
        enc_b = chunked(np.ascontiguousarray(enc_t[n].T))  # (P, JC, T)
        dec_b = chunked(np.ascontiguousarray(dec_t[n].T))  # (P, JC, U)
        db = np.empty((P, JC, DEC_COLS), np.float32)
        db[:, :, OFF_WDEC:OFF_WDEC + J] = w_dec_b
        db[:, :, OFF_DEC:OFF_DEC + U] = dec_b
        db[:, :, OFF_BSUM:OFF_BSUM + 1] = bsum_b
        in_maps.append({
            "w_enc16": w_enc_b.astype(bf),
            "dec_blob": db.astype(bf),
            "enc_t16": enc_b.astype(bf),
            "w_out01": w_out01,
            "w_out23": w_out23,
            "brow_r": brow,
            "b_out_bc": b_out_bc,
        })
    return in_maps


def get_nc():
    if "nc" not in _CACHE:
        _CACHE["nc"] = _build_nc()
    return _CACHE["nc"]


def run_on_hw(in_maps, trace=False):
    from concourse.bass_utils import run_bass_kernel_spmd

    nc = get_nc()
    return run_bass_kernel_spmd(nc, in_maps, core_ids=list(range(N)), trace=trace)


def kernel(encoder_out, decoder_out, W_enc, b_enc, W_dec, b_dec, W_out, b_out):
    in_maps = _prep_inputs(
        encoder_out, decoder_out, W_enc, b_enc, W_dec, b_dec, W_out, b_out
    )
    res = run_on_hw(in_maps)
    out = np.stack(
        [np.asarray(res.results[i]["out"], dtype=np.float32) for i in range(N)],
        axis=0,
    )
    return out.reshape(N, T, U, V)
